# revision 32
# baseline (speedup 1.0000x reference)
"""Cross-attention kernel for 8 trn2 NeuronCores.

Reference computation (per batch b of 16):
  q = Wq @ x, k = Wk @ y, v = Wv @ y          (1x1 convs as channel matmuls)
  q,k l2-normalized over the SPATIAL axis (per (h,d) row)
  sim = 10 * q^T k per head; attn = softmax_j(sim); o = attn @ v^T
  out = Wo @ o + b

Sharding: data-parallel over batch, 2 batches per core, weights replicated.

v2 design (vs baseline): transposed PV + multi-engine exp.
  - S_T[j,i] per (head, jc): one 1024-col matmul, contraction 64.
  - exp(10*sim) split across engines: most tiles on ACT (exact); a few per
    head computed on Pool+DVE via a degree-2 minimax polynomial
    (et = (sq*(x+h))^2 + k, |rel err| <= 3.2% at range edges, end-to-end
    contribution ~0.3% because logits are tiny: |x| <= 0.65).
  - PV transposed: lhsT = et[:, ic*128:+128] (stationary), rhs = vt[j, 65]
    (64 v cols + ones col) -> ot[i, 8ic x 65]; col 64 of each 65-block is
    the softmax denominator for that i — per-PARTITION scalars, so the
    normalization is reciprocal [128,8] + 8 tensor_scalar [128,64] (no
    partition_broadcast / DMA-reshape machinery).  Halves PV's PE time.
  - o back-transposed via PE matmuls against an identity rhs (f32 out into
    the shared acc PSUM pool), then Pool copies to SBUF f16 for zproj.
  - q,k scaled INDEPENDENTLY (1/sqrt(N*uq), 1/sqrt(N*uk)) so each proj
    PSUM tile drains without waiting for the other's stats.
  - elementwise work spread over Pool (0.833ns/el, no init cost) and DVE;
    ACT does only exp.
"""

import sys

import numpy as np

if "/opt/trn_rl_repo" not in sys.path:
    sys.path.insert(0, "/opt/trn_rl_repo")

NB = 2        # batches per core
C = 256       # channels
N = 1024      # spatial (32*32)
HEADS = 4
DH = 64
HID = 256
NCORES = 8
MAGIC = 0x5F3759DF  # Quake fast inverse-sqrt seed

# degree-2 fit of e^x on [-0.8, 0.8], relative-error weighted LSQ
C2, C1, C0 = 0.46848915, 1.05618165, 1.00892716
PH = C1 / (2.0 * C2)
PK = C0 - C1 * C1 / (4.0 * C2)
SQ = float(np.sqrt(C2))

# which jc chunks (per head) go through the poly path instead of ACT
POLY_JCS = ()

_CACHE = {}


def _quake_rsqrt(nc, pool, p_ap, out_ap, final_scale):
    """out = rsqrt(p) * final_scale for [128,1] fp32 APs, DVE-only.

    Quake seed + 2 Newton iterations (rel err ~1e-7), no ACT table needed.
    """
    from concourse import mybir

    i32 = mybir.dt.int32
    alu = mybir.AluOpType
    t = pool.tile([128, 1], mybir.dt.float32, tag="qk_rs_t", bufs=4)
    r = pool.tile([128, 1], mybir.dt.float32, tag="qk_rs_r", bufs=4)
    a = pool.tile([128, 1], mybir.dt.float32, tag="qk_rs_a", bufs=4)
    nc.vector.tensor_scalar(t.bitcast(i32), p_ap.bitcast(i32), 1, None,
                            alu.logical_shift_right)
    nc.vector.tensor_scalar(r.bitcast(i32), t.bitcast(i32), -1, MAGIC,
                            alu.mult, alu.add)
    nc.vector.scalar_tensor_tensor(a[:], r[:], r[:, 0:1], p_ap,
                                   alu.mult, alu.mult)
    nc.vector.tensor_scalar(a[:], a[:], -0.5, 1.5, alu.mult, alu.add)
    nc.vector.tensor_scalar(t[:], a[:], r[:, 0:1], None, alu.mult)
    nc.vector.scalar_tensor_tensor(a[:], t[:], t[:, 0:1], p_ap,
                                   alu.mult, alu.mult)
    nc.vector.tensor_scalar(a[:], a[:], -0.5, 1.5, alu.mult, alu.add)
    nc.vector.tensor_scalar(out_ap, a[:], t[:, 0:1], final_scale,
                            alu.mult, alu.mult)


def _build_nc():
    from contextlib import ExitStack

    import concourse.tile as tile
    from concourse import bacc, mybir

    f32 = mybir.dt.float32
    f16 = mybir.dt.float16
    alu = mybir.AluOpType
    EXP = mybir.ActivationFunctionType.Exp

    nc = bacc.Bacc("TRN2", target_bir_lowering=False)

    xin = nc.dram_tensor("x", [NB, C, N], f16, kind="ExternalInput")
    yin = nc.dram_tensor("y", [NB, C, N], f16, kind="ExternalInput")
    wq = nc.dram_tensor("wq_t", [C, HID], f16, kind="ExternalInput")
    wk = nc.dram_tensor("wk_t", [C, HID], f16, kind="ExternalInput")
    wv = nc.dram_tensor("wv_t", [C, HID], f16, kind="ExternalInput")
    wo = nc.dram_tensor("wo_t", [HID, C], f16, kind="ExternalInput")
    bo = nc.dram_tensor("b_out", [2, 128, 1], f32, kind="ExternalInput")
    idin = nc.dram_tensor("ident", [128, 128], f16, kind="ExternalInput")
    out = nc.dram_tensor("out", [NB, C, N], f32, kind="ExternalOutput")

    with tile.TileContext(nc) as tc, ExitStack() as ctx:
        consts = ctx.enter_context(tc.tile_pool(name="consts", bufs=1))
        big = ctx.enter_context(tc.tile_pool(name="big", bufs=2))
        sm = ctx.enter_context(tc.tile_pool(name="sm", bufs=4))
        ps = ctx.enter_context(tc.tile_pool(name="ps", bufs=2, space="PSUM"))

        # ---- input + weight loads (batch-0 y first: critical path) ----
        wq_sb = consts.tile([128, 2, HID], f16, tag="wq")
        wk_sb = consts.tile([128, 2, HID], f16, tag="wk")
        wv_sb = consts.tile([128, 2, HID], f16, tag="wv")
        wo_sb = consts.tile([128, 2, C], f16, tag="wo")
        b_sb = consts.tile([128, 2, 1], f32, tag="bo")
        id_sb = consts.tile([128, 128], f16, tag="ident")
        onec = consts.tile([128, 1], f16, tag="onec")
        nc.gpsimd.memset(onec[:], 1.0)
        # warm the ACT exp table while input DMAs are in flight
        warm = sm.tile([128, 1], f32, tag="warm", bufs=1)
        nc.vector.memset(warm[:], 0.0)
        nc.scalar.activation(out=warm[:], in_=warm[:], func=EXP, scale=1.0)
        xts, yts = [], []
        for nb in range(NB):
            xt = big.tile([128, 2, N], f16, tag="xt", bufs=2)
            yt = big.tile([128, 2, N], f16, tag="yt", bufs=2)
            xts.append(xt)
            yts.append(yt)
        nc.sync.dma_start(out=yts[0][:, 0], in_=yin[0, 0:128])
        nc.sync.dma_start(out=wk_sb[:], in_=wk.rearrange("(kc p) n -> p kc n", p=128))
        nc.sync.dma_start(out=yts[0][:, 1], in_=yin[0, 128:256])
        nc.sync.dma_start(out=wq_sb[:], in_=wq.rearrange("(kc p) n -> p kc n", p=128))
        nc.sync.dma_start(out=xts[0][:, 0], in_=xin[0, 0:128])
        nc.sync.dma_start(out=xts[0][:, 1], in_=xin[0, 128:256])
        nc.sync.dma_start(out=wv_sb[:], in_=wv.rearrange("(kc p) n -> p kc n", p=128))
        nc.sync.dma_start(out=wo_sb[:], in_=wo.rearrange("(kc p) n -> p kc n", p=128))
        nc.sync.dma_start(out=b_sb[:], in_=bo.rearrange("kc p n -> p kc n"))
        nc.sync.dma_start(out=id_sb[:], in_=idin[:, :])
        nc.sync.dma_start(out=yts[1][:], in_=yin[1].rearrange("(kc p) n -> p kc n", p=128))
        nc.sync.dma_start(out=xts[1][:], in_=xin[1].rearrange("(kc p) n -> p kc n", p=128))

        # per-batch persistent SBUF tiles
        qns, kns, otrs, onorms = [], [], [], []
        for nb in range(NB):
            qns.append(big.tile([128, 2, N], f16, tag="qn", bufs=2, name=f"qn{nb}"))
            kns.append(big.tile([128, 2, N], f16, tag="kn", bufs=2, name=f"kn{nb}"))
            otrs.append(big.tile([128, 2, N], f16, tag="otr", bufs=2, name=f"otr{nb}"))
            onorms.append(big.tile([128, 8, HID], f16, tag="onorm", bufs=2, name=f"onorm{nb}"))
        vts = [[], []]

        def proj_one(nb, mc, w_sb, src, dst, fast=False):
            """dst[:, mc, :] = (w_sb chunk mc)^T @ src, scaled by 1/sqrt(N*u).

            Two [128,512] acc-ring tiles (not the st ring!) so the stats ->
            rsqrt -> drain chain never stalls the S_T/exp stream."""
            raw = None if fast else big.tile([128, N], f16, tag="praw", bufs=2)
            pps = []
            for ih in range(2):
                pp = ps.tile([128, 512], f32, tag="acc", bufs=2, name=f"pp{ih}")
                for kc in range(2):
                    nc.tensor.matmul(
                        pp[:],
                        w_sb[:, kc, mc * 128:(mc + 1) * 128],
                        src[:, kc, ih * 512:(ih + 1) * 512],
                        start=(kc == 0), stop=(kc == 1))
                pps.append(pp)
                if not fast:
                    # drain PSUM immediately (frees the acc slot in ~0.7us);
                    # stats/scale run from SBUF afterwards
                    nc.vector.tensor_copy(raw[:, ih * 512:(ih + 1) * 512], pp[:])
            st2 = sm.tile([128, 2, 6], f32, tag="st2", bufs=4)
            mv = sm.tile([128, 2], f32, tag="mv", bufs=4)
            for sub in range(2):
                nc.vector.bn_stats(out=st2[:, sub, :],
                                   in_=pps[sub][:] if fast
                                   else raw[:, sub * 512:(sub + 1) * 512])
            nc.vector.bn_aggr(out=mv[:], in_=st2[:])
            u = sm.tile([128, 1], f32, tag="u", bufs=4)
            nc.vector.scalar_tensor_tensor(u[:], mv[:, 0:1], mv[:, 0:1],
                                           mv[:, 1:2], alu.mult, alu.add)
            sc = sm.tile([128, 1], f32, tag="sc", bufs=4)
            _quake_rsqrt(nc, sm, u[:], sc[:], 1.0 / float(np.sqrt(N)))
            if fast:
                for ih in range(2):
                    nc.vector.tensor_scalar(dst[:, mc, ih * 512:(ih + 1) * 512],
                                            pps[ih][:], sc[:, 0:1], None, alu.mult)
            else:
                nc.gpsimd.tensor_scalar(dst[:, mc, :], raw[:], sc[:, 0:1], None, alu.mult)

        def proj_v(nb, jcs):
            for jc in jcs:
                vp = ps.tile([128, 512], f32, tag="acc", bufs=2)
                for kc in range(2):
                    nc.tensor.matmul(
                        vp[:, 0:HID],
                        yts[nb][:, kc, jc * 128:(jc + 1) * 128],
                        wv_sb[:, kc, :],
                        start=(kc == 0), stop=(kc == 1))
                vt = big.tile([128, 4, 65], f16, tag="vt", bufs=16)
                nc.vector.tensor_copy(vt[:, :, 0:64],
                                      vp[:, 0:HID].rearrange("p (h d) -> p h d", h=4))
                nc.gpsimd.memset(vt[:, :, 64:65], 1.0)
                vts[nb].append(vt)

        def stexp(nb, h, jc):
            """S_T + exp for one (head, jc) tile; returns the et tile."""
            hp, hr = h // 2, 64 * (h % 2)
            qn, kn = qns[nb], kns[nb]
            st = ps.tile([128, N], f32, tag="st", bufs=2)
            for ih in range(2):
                nc.tensor.matmul(
                    st[:, ih * 512:(ih + 1) * 512],
                    kn[hr:hr + 64, hp, jc * 128:(jc + 1) * 128],
                    qn[hr:hr + 64, hp, ih * 512:(ih + 1) * 512],
                    start=True, stop=True)
            et = big.tile([128, N], f16, tag="et", bufs=26)
            if jc in POLY_JCS:
                # et = (SQ*(10*sim + PH))^2 + PK  (deg-2 exp fit)
                t16 = big.tile([128, N], f16, tag="t16", bufs=2)
                u16 = big.tile([128, N], f16, tag="u16", bufs=2)
                nc.vector.tensor_scalar(t16[:], st[:], 10.0 * SQ, SQ * PH,
                                        alu.mult, alu.add)
                nc.gpsimd.tensor_tensor(u16[:], t16[:], t16[:], alu.mult)
                nc.gpsimd.tensor_scalar(et[:], u16[:], PK, None, alu.add)
            else:
                nc.scalar.activation(out=et[:], in_=st[:], func=EXP, scale=10.0)
            return et

        def pv_chunk(nb, h, ets, ota, otb, ics):
            # PV: ic-outer so each PSUM accumulation group (over jc) completes
            # before the next one starts — start=True clears the whole bank's
            # has_written bits, so interleaved groups in a bank lose their
            # first contribution.  Col 64 of each 65-block (from vt's ones
            # column) accumulates the softmax denominator.
            for ic in ics:
                ot = ota if ic < 4 else otb
                for jc in range(8):
                    nc.tensor.matmul(
                        ot[:, ic % 4, :],
                        ets[jc][:, ic * 128:(ic + 1) * 128],
                        vts[nb][jc][:, h, :],
                        start=(jc == 0), stop=(jc == 7))

        def norm_head(nb, h, ota, otb, act=False):
            r8 = sm.tile([128, 8], f32, tag="r8", bufs=4)
            nc.vector.reciprocal(r8[:, 0:4], ota[:, :, 64])
            nc.vector.reciprocal(r8[:, 4:8], otb[:, :, 64])
            oraw = big.tile([128, 8, 64], f16, tag="oraw", bufs=2)
            if act:
                nc.scalar.copy(oraw[:, 0:4, :], ota[:, :, 0:64])
                nc.scalar.copy(oraw[:, 4:8, :], otb[:, :, 0:64])
            else:
                nc.vector.tensor_copy(oraw[:, 0:4, :], ota[:, :, 0:64])
                nc.vector.tensor_copy(oraw[:, 4:8, :], otb[:, :, 0:64])
            on = onorms[nb]
            for ic in range(8):
                nc.gpsimd.tensor_scalar(on[:, ic, h * 64:(h + 1) * 64],
                                        oraw[:, ic, :], r8[:, ic:ic + 1],
                                        None, alu.mult)

        def trans_o(nb, kcs=(0, 1), act=False):
            on, otr = onorms[nb], otrs[nb]
            for kc in kcs:
                for ih in range(2):
                    tp = ps.tile([128, 512], f32, tag="acc", bufs=2)
                    for qt in range(4):
                        ic = ih * 4 + qt
                        nc.tensor.matmul(
                            tp[:, qt * 128:(qt + 1) * 128],
                            on[:, ic, kc * 128:(kc + 1) * 128],
                            id_sb[:],
                            start=True, stop=True)
                    if act:
                        nc.scalar.copy(otr[:, kc, ih * 512:(ih + 1) * 512], tp[:])
                    else:
                        nc.vector.tensor_copy(otr[:, kc, ih * 512:(ih + 1) * 512], tp[:])

        def zproj(nb):
            otr = otrs[nb]
            for mc in range(2):
                zs = big.tile([128, N], f32, tag="zs", bufs=2)
                for ih in range(2):
                    zp = ps.tile([128, 512], f32, tag="acc", bufs=2)
                    for kc in range(2):
                        nc.tensor.matmul(
                            zp[:],
                            wo_sb[:, kc, mc * 128:(mc + 1) * 128],
                            otr[:, kc, ih * 512:(ih + 1) * 512],
                            start=(kc == 0), stop=(kc == 1))
                    nc.vector.tensor_scalar(zs[:, ih * 512:(ih + 1) * 512], zp[:],
                                            b_sb[:, mc, 0:1], None, alu.add)
                    nc.sync.dma_start(
                        out=out[nb, mc * 128:(mc + 1) * 128, ih * 512:(ih + 1) * 512],
                        in_=zs[:, ih * 512:(ih + 1) * 512])

        # ---- software-pipelined emission: (st+exp) of head slot s runs
        # while PV of slot s-1 accumulates; projections/transposes/zproj of
        # the other batch ride between slots. ----
        HEAD_SEQ = [(0, 0), (0, 1), (0, 2), (0, 3), (1, 0), (1, 1), (1, 2), (1, 3)]
        # extra work interleaved after each slot's st/exp stream (one
        # projection per slot so PSUM-drain latency never queues up):
        PRE = {
            0: lambda: proj_one(0, 1, wk_sb, yts[0], kns[0]),
            1: lambda: proj_one(0, 1, wq_sb, xts[0], qns[0]),
            2: lambda: proj_one(1, 0, wk_sb, yts[1], kns[1]),
            3: lambda: (proj_v(1, range(0, 4)),
                        proj_one(1, 0, wq_sb, xts[1], qns[1])),
            4: lambda: (proj_v(1, range(4, 8)),
                        proj_one(1, 1, wk_sb, yts[1], kns[1])),
            5: lambda: (proj_one(1, 1, wq_sb, xts[1], qns[1]),
                        trans_o(0)),
            6: lambda: zproj(0),
            7: lambda: trans_o(1, kcs=(0,)),
        }
        proj_one(0, 0, wk_sb, yts[0], kns[0], fast=True)
        proj_one(0, 0, wq_sb, xts[0], qns[0], fast=True)
        prev = None  # (nb, h, ets, ot, dn)
        for s, (nb, h) in enumerate(HEAD_SEQ):
            ets = []
            for jc in range(8):
                ets.append(stexp(nb, h, jc))
                if s == 0 and jc == 1:
                    proj_v(0, range(8))
                # weave the previous head's PV between st pairs (coarse
                # chunks so a ring-stalled st never blocks ready PV work
                # for long, but ACT stays fed across the slot boundary)
                if prev is not None and jc == 1:
                    pv_chunk(*prev, ics=range(0, 4))
                if prev is not None and jc == 3:
                    pv_chunk(*prev, ics=range(4, 8))
                if prev is not None and jc == 4:
                    norm_head(prev[0], prev[1], prev[3], prev[4])
            ota = ps.tile([128, 4, 65], f32, tag="ota", bufs=1, name=f"ota{s}")
            otb = ps.tile([128, 4, 65], f32, tag="otb", bufs=1, name=f"otb{s}")
            prev = (nb, h, ets, ota, otb)
            if s in PRE:
                PRE[s]()
        pv_chunk(*prev, ics=range(8))
        norm_head(prev[0], prev[1], prev[3], prev[4], act=True)
        trans_o(1, kcs=(1,), act=True)
        zproj(1)
        del onec

    nc.finalize()
    return nc


def _get_nc():
    if "nc" not in _CACHE:
        _CACHE["nc"] = _build_nc()
    return _CACHE["nc"]


def kernel(x, y, w_qkv, w_out, b_out):
    from concourse.bass_utils import run_bass_kernel_spmd

    nc = _get_nc()

    x = np.asarray(x, dtype=np.float32).reshape(16, C, N).astype(np.float16)
    y = np.asarray(y, dtype=np.float32).reshape(16, C, N).astype(np.float16)
    w_qkv = np.asarray(w_qkv, dtype=np.float32)
    wq_t = np.ascontiguousarray(w_qkv[0:HID].T).astype(np.float16)
    wk_t = np.ascontiguousarray(w_qkv[HID:2 * HID].T).astype(np.float16)
    wv_t = np.ascontiguousarray(w_qkv[2 * HID:3 * HID].T).astype(np.float16)
    wo_t = np.ascontiguousarray(np.asarray(w_out, dtype=np.float32).T).astype(np.float16)
    bo = np.ascontiguousarray(
        np.asarray(b_out, dtype=np.float32).reshape(2, 128, 1))
    ident = np.eye(128, dtype=np.float16)

    in_maps = []
    for c in range(NCORES):
        in_maps.append({
            "x": np.ascontiguousarray(x[c * NB:(c + 1) * NB]),
            "y": np.ascontiguousarray(y[c * NB:(c + 1) * NB]),
            "wq_t": wq_t, "wk_t": wk_t, "wv_t": wv_t, "wo_t": wo_t,
            "b_out": bo, "ident": ident,
        })

    res = run_bass_kernel_spmd(nc, in_maps, list(range(NCORES)))
    full = np.concatenate([res.results[i]["out"] for i in range(NCORES)], axis=0)
    return full.reshape(16, C, 32, 32)


# revision 34
# speedup vs baseline: 1.0006x; 1.0006x over previous
"""Cross-attention kernel for 8 trn2 NeuronCores.

Reference computation (per batch b of 16):
  q = Wq @ x, k = Wk @ y, v = Wv @ y          (1x1 convs as channel matmuls)
  q,k l2-normalized over the SPATIAL axis (per (h,d) row)
  sim = 10 * q^T k per head; attn = softmax_j(sim); o = attn @ v^T
  out = Wo @ o + b

Sharding: data-parallel over batch, 2 batches per core, weights replicated.

v2 design (vs baseline): transposed PV + multi-engine exp.
  - S_T[j,i] per (head, jc): one 1024-col matmul, contraction 64.
  - exp(10*sim) split across engines: most tiles on ACT (exact); a few per
    head computed on Pool+DVE via a degree-2 minimax polynomial
    (et = (sq*(x+h))^2 + k, |rel err| <= 3.2% at range edges, end-to-end
    contribution ~0.3% because logits are tiny: |x| <= 0.65).
  - PV transposed: lhsT = et[:, ic*128:+128] (stationary), rhs = vt[j, 65]
    (64 v cols + ones col) -> ot[i, 8ic x 65]; col 64 of each 65-block is
    the softmax denominator for that i — per-PARTITION scalars, so the
    normalization is reciprocal [128,8] + 8 tensor_scalar [128,64] (no
    partition_broadcast / DMA-reshape machinery).  Halves PV's PE time.
  - o back-transposed via PE matmuls against an identity rhs (f32 out into
    the shared acc PSUM pool), then Pool copies to SBUF f16 for zproj.
  - q,k scaled INDEPENDENTLY (1/sqrt(N*uq), 1/sqrt(N*uk)) so each proj
    PSUM tile drains without waiting for the other's stats.
  - elementwise work spread over Pool (0.833ns/el, no init cost) and DVE;
    ACT does only exp.
"""

import sys

import numpy as np

if "/opt/trn_rl_repo" not in sys.path:
    sys.path.insert(0, "/opt/trn_rl_repo")

NB = 2        # batches per core
C = 256       # channels
N = 1024      # spatial (32*32)
HEADS = 4
DH = 64
HID = 256
NCORES = 8
MAGIC = 0x5F3759DF  # Quake fast inverse-sqrt seed

# degree-2 fit of e^x on [-0.8, 0.8], relative-error weighted LSQ
C2, C1, C0 = 0.46848915, 1.05618165, 1.00892716
PH = C1 / (2.0 * C2)
PK = C0 - C1 * C1 / (4.0 * C2)
SQ = float(np.sqrt(C2))

# which jc chunks (per head) go through the poly path instead of ACT
POLY_JCS = ()

_CACHE = {}


def _quake_rsqrt(nc, pool, p_ap, out_ap, final_scale):
    """out = rsqrt(p) * final_scale for [128,1] fp32 APs, DVE-only.

    Quake seed + 2 Newton iterations (rel err ~1e-7), no ACT table needed.
    """
    from concourse import mybir

    i32 = mybir.dt.int32
    alu = mybir.AluOpType
    t = pool.tile([128, 1], mybir.dt.float32, tag="qk_rs_t", bufs=4)
    r = pool.tile([128, 1], mybir.dt.float32, tag="qk_rs_r", bufs=4)
    a = pool.tile([128, 1], mybir.dt.float32, tag="qk_rs_a", bufs=4)
    nc.vector.tensor_scalar(t.bitcast(i32), p_ap.bitcast(i32), 1, None,
                            alu.logical_shift_right)
    nc.vector.tensor_scalar(r.bitcast(i32), t.bitcast(i32), -1, MAGIC,
                            alu.mult, alu.add)
    nc.vector.scalar_tensor_tensor(a[:], r[:], r[:, 0:1], p_ap,
                                   alu.mult, alu.mult)
    nc.vector.tensor_scalar(a[:], a[:], -0.5, 1.5, alu.mult, alu.add)
    nc.vector.tensor_scalar(t[:], a[:], r[:, 0:1], None, alu.mult)
    nc.vector.scalar_tensor_tensor(a[:], t[:], t[:, 0:1], p_ap,
                                   alu.mult, alu.mult)
    nc.vector.tensor_scalar(a[:], a[:], -0.5, 1.5, alu.mult, alu.add)
    nc.vector.tensor_scalar(out_ap, a[:], t[:, 0:1], final_scale,
                            alu.mult, alu.mult)


def _build_nc():
    from contextlib import ExitStack

    import concourse.tile as tile
    from concourse import bacc, mybir

    f32 = mybir.dt.float32
    f16 = mybir.dt.float16
    alu = mybir.AluOpType
    EXP = mybir.ActivationFunctionType.Exp

    nc = bacc.Bacc("TRN2", target_bir_lowering=False)

    xin = nc.dram_tensor("x", [NB, C, N], f16, kind="ExternalInput")
    yin = nc.dram_tensor("y", [NB, C, N], f16, kind="ExternalInput")
    wq = nc.dram_tensor("wq_t", [C, HID], f16, kind="ExternalInput")
    wk = nc.dram_tensor("wk_t", [C, HID], f16, kind="ExternalInput")
    wv = nc.dram_tensor("wv_t", [C, HID], f16, kind="ExternalInput")
    wo = nc.dram_tensor("wo_t", [HID, C], f16, kind="ExternalInput")
    bo = nc.dram_tensor("b_out", [2, 128, 1], f32, kind="ExternalInput")
    idin = nc.dram_tensor("ident", [128, 128], f16, kind="ExternalInput")
    out = nc.dram_tensor("out", [NB, C, N], f32, kind="ExternalOutput")

    with tile.TileContext(nc) as tc, ExitStack() as ctx:
        consts = ctx.enter_context(tc.tile_pool(name="consts", bufs=1))
        big = ctx.enter_context(tc.tile_pool(name="big", bufs=2))
        sm = ctx.enter_context(tc.tile_pool(name="sm", bufs=4))
        ps = ctx.enter_context(tc.tile_pool(name="ps", bufs=2, space="PSUM"))

        # ---- input + weight loads (batch-0 y first: critical path) ----
        wq_sb = consts.tile([128, 2, HID], f16, tag="wq")
        wk_sb = consts.tile([128, 2, HID], f16, tag="wk")
        wv_sb = consts.tile([128, 2, HID], f16, tag="wv")
        wo_sb = consts.tile([128, 2, C], f16, tag="wo")
        b_sb = consts.tile([128, 2, 1], f32, tag="bo")
        id_sb = consts.tile([128, 128], f16, tag="ident")
        onec = consts.tile([128, 1], f16, tag="onec")
        nc.gpsimd.memset(onec[:], 1.0)
        # warm the ACT exp table while input DMAs are in flight
        warm = sm.tile([128, 1], f32, tag="warm", bufs=1)
        nc.vector.memset(warm[:], 0.0)
        nc.scalar.activation(out=warm[:], in_=warm[:], func=EXP, scale=1.0)
        xts, yts = [], []
        for nb in range(NB):
            xt = big.tile([128, 2, N], f16, tag="xt", bufs=2)
            yt = big.tile([128, 2, N], f16, tag="yt", bufs=2)
            xts.append(xt)
            yts.append(yt)
        nc.sync.dma_start(out=yts[0][:, 0], in_=yin[0, 0:128])
        nc.sync.dma_start(out=wk_sb[:], in_=wk.rearrange("(kc p) n -> p kc n", p=128))
        nc.sync.dma_start(out=yts[0][:, 1], in_=yin[0, 128:256])
        nc.sync.dma_start(out=wq_sb[:], in_=wq.rearrange("(kc p) n -> p kc n", p=128))
        nc.sync.dma_start(out=xts[0][:, 0], in_=xin[0, 0:128])
        nc.sync.dma_start(out=xts[0][:, 1], in_=xin[0, 128:256])
        nc.sync.dma_start(out=wv_sb[:], in_=wv.rearrange("(kc p) n -> p kc n", p=128))
        nc.sync.dma_start(out=wo_sb[:], in_=wo.rearrange("(kc p) n -> p kc n", p=128))
        nc.sync.dma_start(out=b_sb[:], in_=bo.rearrange("kc p n -> p kc n"))
        nc.sync.dma_start(out=id_sb[:], in_=idin[:, :])
        nc.sync.dma_start(out=yts[1][:], in_=yin[1].rearrange("(kc p) n -> p kc n", p=128))
        nc.sync.dma_start(out=xts[1][:], in_=xin[1].rearrange("(kc p) n -> p kc n", p=128))

        # per-batch persistent SBUF tiles
        qns, kns, otrs, onorms = [], [], [], []
        for nb in range(NB):
            qns.append(big.tile([128, 2, N], f16, tag="qn", bufs=2, name=f"qn{nb}"))
            kns.append(big.tile([128, 2, N], f16, tag="kn", bufs=2, name=f"kn{nb}"))
            otrs.append(big.tile([128, 2, N], f16, tag="otr", bufs=2, name=f"otr{nb}"))
            onorms.append(big.tile([128, 8, HID], f16, tag="onorm", bufs=2, name=f"onorm{nb}"))
        vts = [[], []]

        def proj_one(nb, mc, w_sb, src, dst, fast=False):
            """dst[:, mc, :] = (w_sb chunk mc)^T @ src, scaled by 1/sqrt(N*u).

            Two [128,512] acc-ring tiles (not the st ring!) so the stats ->
            rsqrt -> drain chain never stalls the S_T/exp stream."""
            raw = None if fast else big.tile([128, N], f16, tag="praw", bufs=2)
            pps = []
            for ih in range(2):
                pp = ps.tile([128, 512], f32, tag="acc", bufs=2, name=f"pp{ih}")
                for kc in range(2):
                    nc.tensor.matmul(
                        pp[:],
                        w_sb[:, kc, mc * 128:(mc + 1) * 128],
                        src[:, kc, ih * 512:(ih + 1) * 512],
                        start=(kc == 0), stop=(kc == 1))
                pps.append(pp)
                if not fast:
                    # drain PSUM immediately (frees the acc slot in ~0.7us);
                    # stats/scale run from SBUF afterwards
                    nc.vector.tensor_copy(raw[:, ih * 512:(ih + 1) * 512], pp[:])
            u = sm.tile([128, 1], f32, tag="u", bufs=4)
            if fast:
                # sum-of-squares via ACT Square+accumulator: ACT is idle
                # before the first exp, and this shortens the serial DVE
                # chain on the startup critical path
                SQF = mybir.ActivationFunctionType.Square
                junk2 = sm.tile([128, 512], f16, tag="junk2", bufs=2)
                us = sm.tile([128, 2], f32, tag="us", bufs=2)
                for ih in range(2):
                    nc.scalar.activation(out=junk2[:], in_=pps[ih][:], func=SQF,
                                         accum_out=us[:, ih:ih + 1])
                nc.vector.tensor_scalar(u[:], us[:, 0:1], us[:, 1:2], None,
                                        alu.add)
            else:
                st2 = sm.tile([128, 2, 6], f32, tag="st2", bufs=4)
                mv = sm.tile([128, 2], f32, tag="mv", bufs=4)
                for sub in range(2):
                    nc.vector.bn_stats(out=st2[:, sub, :],
                                       in_=raw[:, sub * 512:(sub + 1) * 512])
                nc.vector.bn_aggr(out=mv[:], in_=st2[:])
                nc.vector.scalar_tensor_tensor(u[:], mv[:, 0:1], mv[:, 0:1],
                                               mv[:, 1:2], alu.mult, alu.add)
            sc = sm.tile([128, 1], f32, tag="sc", bufs=4)
            _quake_rsqrt(nc, sm, u[:], sc[:],
                         1.0 if fast else 1.0 / float(np.sqrt(N)))
            if fast:
                for ih in range(2):
                    nc.vector.tensor_scalar(dst[:, mc, ih * 512:(ih + 1) * 512],
                                            pps[ih][:], sc[:, 0:1], None, alu.mult)
            else:
                nc.gpsimd.tensor_scalar(dst[:, mc, :], raw[:], sc[:, 0:1], None, alu.mult)

        def proj_v(nb, jcs):
            for jc in jcs:
                vp = ps.tile([128, 512], f32, tag="acc", bufs=2)
                for kc in range(2):
                    nc.tensor.matmul(
                        vp[:, 0:HID],
                        yts[nb][:, kc, jc * 128:(jc + 1) * 128],
                        wv_sb[:, kc, :],
                        start=(kc == 0), stop=(kc == 1))
                vt = big.tile([128, 4, 65], f16, tag="vt", bufs=16)
                nc.vector.tensor_copy(vt[:, :, 0:64],
                                      vp[:, 0:HID].rearrange("p (h d) -> p h d", h=4))
                nc.gpsimd.memset(vt[:, :, 64:65], 1.0)
                vts[nb].append(vt)

        def stexp(nb, h, jc):
            """S_T + exp for one (head, jc) tile; returns the et tile."""
            hp, hr = h // 2, 64 * (h % 2)
            qn, kn = qns[nb], kns[nb]
            st = ps.tile([128, N], f32, tag="st", bufs=2)
            for ih in range(2):
                nc.tensor.matmul(
                    st[:, ih * 512:(ih + 1) * 512],
                    kn[hr:hr + 64, hp, jc * 128:(jc + 1) * 128],
                    qn[hr:hr + 64, hp, ih * 512:(ih + 1) * 512],
                    start=True, stop=True)
            et = big.tile([128, N], f16, tag="et", bufs=26)
            if jc in POLY_JCS:
                # et = (SQ*(10*sim + PH))^2 + PK  (deg-2 exp fit)
                t16 = big.tile([128, N], f16, tag="t16", bufs=2)
                u16 = big.tile([128, N], f16, tag="u16", bufs=2)
                nc.vector.tensor_scalar(t16[:], st[:], 10.0 * SQ, SQ * PH,
                                        alu.mult, alu.add)
                nc.gpsimd.tensor_tensor(u16[:], t16[:], t16[:], alu.mult)
                nc.gpsimd.tensor_scalar(et[:], u16[:], PK, None, alu.add)
            else:
                nc.scalar.activation(out=et[:], in_=st[:], func=EXP, scale=10.0)
            return et

        def pv_chunk(nb, h, ets, ota, otb, ics):
            # PV: ic-outer so each PSUM accumulation group (over jc) completes
            # before the next one starts — start=True clears the whole bank's
            # has_written bits, so interleaved groups in a bank lose their
            # first contribution.  Col 64 of each 65-block (from vt's ones
            # column) accumulates the softmax denominator.
            for ic in ics:
                ot = ota if ic < 4 else otb
                for jc in range(8):
                    nc.tensor.matmul(
                        ot[:, ic % 4, :],
                        ets[jc][:, ic * 128:(ic + 1) * 128],
                        vts[nb][jc][:, h, :],
                        start=(jc == 0), stop=(jc == 7))

        def norm_head(nb, h, ota, otb, act=False):
            r8 = sm.tile([128, 8], f32, tag="r8", bufs=4)
            nc.vector.reciprocal(r8[:, 0:4], ota[:, :, 64])
            nc.vector.reciprocal(r8[:, 4:8], otb[:, :, 64])
            oraw = big.tile([128, 8, 64], f16, tag="oraw", bufs=2)
            if act:
                nc.scalar.copy(oraw[:, 0:4, :], ota[:, :, 0:64])
                nc.scalar.copy(oraw[:, 4:8, :], otb[:, :, 0:64])
            else:
                nc.vector.tensor_copy(oraw[:, 0:4, :], ota[:, :, 0:64])
                nc.vector.tensor_copy(oraw[:, 4:8, :], otb[:, :, 0:64])
            on = onorms[nb]
            for ic in range(8):
                nc.gpsimd.tensor_scalar(on[:, ic, h * 64:(h + 1) * 64],
                                        oraw[:, ic, :], r8[:, ic:ic + 1],
                                        None, alu.mult)

        def trans_o(nb, kcs=(0, 1), act=False):
            on, otr = onorms[nb], otrs[nb]
            for kc in kcs:
                for ih in range(2):
                    tp = ps.tile([128, 512], f32, tag="acc", bufs=2)
                    for qt in range(4):
                        ic = ih * 4 + qt
                        nc.tensor.matmul(
                            tp[:, qt * 128:(qt + 1) * 128],
                            on[:, ic, kc * 128:(kc + 1) * 128],
                            id_sb[:],
                            start=True, stop=True)
                    if act:
                        nc.scalar.copy(otr[:, kc, ih * 512:(ih + 1) * 512], tp[:])
                    else:
                        nc.vector.tensor_copy(otr[:, kc, ih * 512:(ih + 1) * 512], tp[:])

        def zproj(nb, act=False):
            IDENT = mybir.ActivationFunctionType.Identity
            otr = otrs[nb]
            for mc in range(2):
                zs = big.tile([128, N], f32, tag="zs", bufs=2)
                for ih in range(2):
                    zp = ps.tile([128, 512], f32, tag="acc", bufs=2)
                    for kc in range(2):
                        nc.tensor.matmul(
                            zp[:],
                            wo_sb[:, kc, mc * 128:(mc + 1) * 128],
                            otr[:, kc, ih * 512:(ih + 1) * 512],
                            start=(kc == 0), stop=(kc == 1))
                    if act:
                        nc.scalar.activation(out=zs[:, ih * 512:(ih + 1) * 512],
                                             in_=zp[:], func=IDENT,
                                             bias=b_sb[:, mc, 0:1], scale=1.0)
                    else:
                        nc.vector.tensor_scalar(zs[:, ih * 512:(ih + 1) * 512], zp[:],
                                                b_sb[:, mc, 0:1], None, alu.add)
                    nc.sync.dma_start(
                        out=out[nb, mc * 128:(mc + 1) * 128, ih * 512:(ih + 1) * 512],
                        in_=zs[:, ih * 512:(ih + 1) * 512])

        # ---- software-pipelined emission: (st+exp) of head slot s runs
        # while PV of slot s-1 accumulates; projections/transposes/zproj of
        # the other batch ride between slots. ----
        HEAD_SEQ = [(0, 0), (0, 1), (0, 2), (0, 3), (1, 0), (1, 1), (1, 2), (1, 3)]
        # extra work interleaved after each slot's st/exp stream (one
        # projection per slot so PSUM-drain latency never queues up):
        PRE = {
            0: lambda: proj_one(0, 1, wk_sb, yts[0], kns[0]),
            1: lambda: proj_one(0, 1, wq_sb, xts[0], qns[0]),
            2: lambda: proj_one(1, 0, wk_sb, yts[1], kns[1]),
            3: lambda: (proj_v(1, range(0, 4)),
                        proj_one(1, 0, wq_sb, xts[1], qns[1])),
            4: lambda: (proj_v(1, range(4, 8)),
                        proj_one(1, 1, wk_sb, yts[1], kns[1])),
            5: lambda: (proj_one(1, 1, wq_sb, xts[1], qns[1]),
                        trans_o(0)),
            6: lambda: zproj(0),
            7: lambda: trans_o(1, kcs=(0,)),
        }
        proj_one(0, 0, wk_sb, yts[0], kns[0], fast=True)
        proj_one(0, 0, wq_sb, xts[0], qns[0], fast=True)
        prev = None  # (nb, h, ets, ot, dn)
        for s, (nb, h) in enumerate(HEAD_SEQ):
            ets = []
            for jc in range(8):
                ets.append(stexp(nb, h, jc))
                if s == 0 and jc == 1:
                    proj_v(0, range(8))
                # weave the previous head's PV between st pairs (coarse
                # chunks so a ring-stalled st never blocks ready PV work
                # for long, but ACT stays fed across the slot boundary)
                if prev is not None and jc == 1:
                    pv_chunk(*prev, ics=range(0, 4))
                if prev is not None and jc == 3:
                    pv_chunk(*prev, ics=range(4, 8))
                if prev is not None and jc == 4:
                    norm_head(prev[0], prev[1], prev[3], prev[4])
            ota = ps.tile([128, 4, 65], f32, tag="ota", bufs=1, name=f"ota{s}")
            otb = ps.tile([128, 4, 65], f32, tag="otb", bufs=1, name=f"otb{s}")
            prev = (nb, h, ets, ota, otb)
            if s in PRE:
                PRE[s]()
        pv_chunk(*prev, ics=range(8))
        norm_head(prev[0], prev[1], prev[3], prev[4], act=True)
        trans_o(1, kcs=(1,), act=True)
        zproj(1)
        del onec

    nc.finalize()
    return nc


def _get_nc():
    if "nc" not in _CACHE:
        _CACHE["nc"] = _build_nc()
    return _CACHE["nc"]


def kernel(x, y, w_qkv, w_out, b_out):
    from concourse.bass_utils import run_bass_kernel_spmd

    nc = _get_nc()

    x = np.asarray(x, dtype=np.float32).reshape(16, C, N).astype(np.float16)
    y = np.asarray(y, dtype=np.float32).reshape(16, C, N).astype(np.float16)
    w_qkv = np.asarray(w_qkv, dtype=np.float32)
    wq_t = np.ascontiguousarray(w_qkv[0:HID].T).astype(np.float16)
    wk_t = np.ascontiguousarray(w_qkv[HID:2 * HID].T).astype(np.float16)
    wv_t = np.ascontiguousarray(w_qkv[2 * HID:3 * HID].T).astype(np.float16)
    wo_t = np.ascontiguousarray(np.asarray(w_out, dtype=np.float32).T).astype(np.float16)
    bo = np.ascontiguousarray(
        np.asarray(b_out, dtype=np.float32).reshape(2, 128, 1))
    ident = np.eye(128, dtype=np.float16)

    in_maps = []
    for c in range(NCORES):
        in_maps.append({
            "x": np.ascontiguousarray(x[c * NB:(c + 1) * NB]),
            "y": np.ascontiguousarray(y[c * NB:(c + 1) * NB]),
            "wq_t": wq_t, "wk_t": wk_t, "wv_t": wv_t, "wo_t": wo_t,
            "b_out": bo, "ident": ident,
        })

    res = run_bass_kernel_spmd(nc, in_maps, list(range(NCORES)))
    full = np.concatenate([res.results[i]["out"] for i in range(NCORES)], axis=0)
    return full.reshape(16, C, 32, 32)


# revision 35
# speedup vs baseline: 1.0013x; 1.0007x over previous
"""Cross-attention kernel for 8 trn2 NeuronCores.

Reference computation (per batch b of 16):
  q = Wq @ x, k = Wk @ y, v = Wv @ y          (1x1 convs as channel matmuls)
  q,k l2-normalized over the SPATIAL axis (per (h,d) row)
  sim = 10 * q^T k per head; attn = softmax_j(sim); o = attn @ v^T
  out = Wo @ o + b

Sharding: data-parallel over batch, 2 batches per core, weights replicated.

v2 design (vs baseline): transposed PV + multi-engine exp.
  - S_T[j,i] per (head, jc): one 1024-col matmul, contraction 64.
  - exp(10*sim) split across engines: most tiles on ACT (exact); a few per
    head computed on Pool+DVE via a degree-2 minimax polynomial
    (et = (sq*(x+h))^2 + k, |rel err| <= 3.2% at range edges, end-to-end
    contribution ~0.3% because logits are tiny: |x| <= 0.65).
  - PV transposed: lhsT = et[:, ic*128:+128] (stationary), rhs = vt[j, 65]
    (64 v cols + ones col) -> ot[i, 8ic x 65]; col 64 of each 65-block is
    the softmax denominator for that i — per-PARTITION scalars, so the
    normalization is reciprocal [128,8] + 8 tensor_scalar [128,64] (no
    partition_broadcast / DMA-reshape machinery).  Halves PV's PE time.
  - o back-transposed via PE matmuls against an identity rhs (f32 out into
    the shared acc PSUM pool), then Pool copies to SBUF f16 for zproj.
  - q,k scaled INDEPENDENTLY (1/sqrt(N*uq), 1/sqrt(N*uk)) so each proj
    PSUM tile drains without waiting for the other's stats.
  - elementwise work spread over Pool (0.833ns/el, no init cost) and DVE;
    ACT does only exp.
"""

import sys

import numpy as np

if "/opt/trn_rl_repo" not in sys.path:
    sys.path.insert(0, "/opt/trn_rl_repo")

NB = 2        # batches per core
C = 256       # channels
N = 1024      # spatial (32*32)
HEADS = 4
DH = 64
HID = 256
NCORES = 8
MAGIC = 0x5F3759DF  # Quake fast inverse-sqrt seed

# degree-2 fit of e^x on [-0.8, 0.8], relative-error weighted LSQ
C2, C1, C0 = 0.46848915, 1.05618165, 1.00892716
PH = C1 / (2.0 * C2)
PK = C0 - C1 * C1 / (4.0 * C2)
SQ = float(np.sqrt(C2))

# which jc chunks (per head) go through the poly path instead of ACT
POLY_JCS = ()

_CACHE = {}


def _quake_rsqrt(nc, pool, p_ap, out_ap, final_scale):
    """out = rsqrt(p) * final_scale for [128,1] fp32 APs, DVE-only.

    Quake seed + 2 Newton iterations (rel err ~1e-7), no ACT table needed.
    """
    from concourse import mybir

    i32 = mybir.dt.int32
    alu = mybir.AluOpType
    t = pool.tile([128, 1], mybir.dt.float32, tag="qk_rs_t", bufs=4)
    r = pool.tile([128, 1], mybir.dt.float32, tag="qk_rs_r", bufs=4)
    a = pool.tile([128, 1], mybir.dt.float32, tag="qk_rs_a", bufs=4)
    nc.vector.tensor_scalar(t.bitcast(i32), p_ap.bitcast(i32), 1, None,
                            alu.logical_shift_right)
    nc.vector.tensor_scalar(r.bitcast(i32), t.bitcast(i32), -1, MAGIC,
                            alu.mult, alu.add)
    nc.vector.scalar_tensor_tensor(a[:], r[:], r[:, 0:1], p_ap,
                                   alu.mult, alu.mult)
    nc.vector.tensor_scalar(a[:], a[:], -0.5, 1.5, alu.mult, alu.add)
    nc.vector.tensor_scalar(t[:], a[:], r[:, 0:1], None, alu.mult)
    nc.vector.scalar_tensor_tensor(a[:], t[:], t[:, 0:1], p_ap,
                                   alu.mult, alu.mult)
    nc.vector.tensor_scalar(a[:], a[:], -0.5, 1.5, alu.mult, alu.add)
    nc.vector.tensor_scalar(out_ap, a[:], t[:, 0:1], final_scale,
                            alu.mult, alu.mult)


def _build_nc():
    from contextlib import ExitStack

    import concourse.tile as tile
    from concourse import bacc, mybir

    f32 = mybir.dt.float32
    f16 = mybir.dt.float16
    alu = mybir.AluOpType
    EXP = mybir.ActivationFunctionType.Exp

    nc = bacc.Bacc("TRN2", target_bir_lowering=False)

    xin = nc.dram_tensor("x", [NB, C, N], f16, kind="ExternalInput")
    yin = nc.dram_tensor("y", [NB, C, N], f16, kind="ExternalInput")
    wq = nc.dram_tensor("wq_t", [C, HID], f16, kind="ExternalInput")
    wk = nc.dram_tensor("wk_t", [C, HID], f16, kind="ExternalInput")
    wv = nc.dram_tensor("wv_t", [C, HID], f16, kind="ExternalInput")
    wo = nc.dram_tensor("wo_t", [HID, C], f16, kind="ExternalInput")
    bo = nc.dram_tensor("b_out", [2, 128, 1], f32, kind="ExternalInput")
    idin = nc.dram_tensor("ident", [128, 128], f16, kind="ExternalInput")
    out = nc.dram_tensor("out", [NB, C, N], f32, kind="ExternalOutput")

    with tile.TileContext(nc) as tc, ExitStack() as ctx:
        consts = ctx.enter_context(tc.tile_pool(name="consts", bufs=1))
        big = ctx.enter_context(tc.tile_pool(name="big", bufs=2))
        sm = ctx.enter_context(tc.tile_pool(name="sm", bufs=4))
        ps = ctx.enter_context(tc.tile_pool(name="ps", bufs=2, space="PSUM"))

        # ---- input + weight loads (batch-0 y first: critical path) ----
        wq_sb = consts.tile([128, 2, HID], f16, tag="wq")
        wk_sb = consts.tile([128, 2, HID], f16, tag="wk")
        wv_sb = consts.tile([128, 2, HID], f16, tag="wv")
        wo_sb = consts.tile([128, 2, C], f16, tag="wo")
        b_sb = consts.tile([128, 2, 1], f32, tag="bo")
        id_sb = consts.tile([128, 128], f16, tag="ident")
        onec = consts.tile([128, 1], f16, tag="onec")
        nc.gpsimd.memset(onec[:], 1.0)
        # warm the ACT exp table while input DMAs are in flight
        warm = sm.tile([128, 1], f32, tag="warm", bufs=1)
        nc.vector.memset(warm[:], 0.0)
        nc.scalar.activation(out=warm[:], in_=warm[:], func=EXP, scale=1.0)
        xts, yts = [], []
        for nb in range(NB):
            xt = big.tile([128, 2, N], f16, tag="xt", bufs=2)
            yt = big.tile([128, 2, N], f16, tag="yt", bufs=2)
            xts.append(xt)
            yts.append(yt)
        nc.sync.dma_start(out=yts[0][:, 0], in_=yin[0, 0:128])
        nc.sync.dma_start(out=wk_sb[:], in_=wk.rearrange("(kc p) n -> p kc n", p=128))
        nc.sync.dma_start(out=yts[0][:, 1], in_=yin[0, 128:256])
        nc.sync.dma_start(out=wq_sb[:], in_=wq.rearrange("(kc p) n -> p kc n", p=128))
        nc.sync.dma_start(out=xts[0][:, 0], in_=xin[0, 0:128])
        nc.sync.dma_start(out=xts[0][:, 1], in_=xin[0, 128:256])
        nc.sync.dma_start(out=wv_sb[:], in_=wv.rearrange("(kc p) n -> p kc n", p=128))
        nc.sync.dma_start(out=wo_sb[:], in_=wo.rearrange("(kc p) n -> p kc n", p=128))
        nc.sync.dma_start(out=b_sb[:], in_=bo.rearrange("kc p n -> p kc n"))
        nc.sync.dma_start(out=id_sb[:], in_=idin[:, :])
        nc.sync.dma_start(out=yts[1][:], in_=yin[1].rearrange("(kc p) n -> p kc n", p=128))
        nc.sync.dma_start(out=xts[1][:], in_=xin[1].rearrange("(kc p) n -> p kc n", p=128))

        # per-batch persistent SBUF tiles
        qns, kns, otrs, onorms = [], [], [], []
        for nb in range(NB):
            qns.append(big.tile([128, 2, N], f16, tag="qn", bufs=2, name=f"qn{nb}"))
            kns.append(big.tile([128, 2, N], f16, tag="kn", bufs=2, name=f"kn{nb}"))
            otrs.append(big.tile([128, 2, N], f16, tag="otr", bufs=2, name=f"otr{nb}"))
            onorms.append(big.tile([128, 8, HID], f16, tag="onorm", bufs=2, name=f"onorm{nb}"))
        vts = [[], []]

        def proj_one(nb, mc, w_sb, src, dst, fast=False):
            """dst[:, mc, :] = (w_sb chunk mc)^T @ src, scaled by 1/sqrt(N*u).

            Two [128,512] acc-ring tiles (not the st ring!) so the stats ->
            rsqrt -> drain chain never stalls the S_T/exp stream."""
            raw = None if fast else big.tile([128, N], f16, tag="praw", bufs=2)
            pps = []
            for ih in range(2):
                pp = ps.tile([128, 512], f32, tag="acc", bufs=2, name=f"pp{ih}")
                for kc in range(2):
                    nc.tensor.matmul(
                        pp[:],
                        w_sb[:, kc, mc * 128:(mc + 1) * 128],
                        src[:, kc, ih * 512:(ih + 1) * 512],
                        start=(kc == 0), stop=(kc == 1))
                pps.append(pp)
                if not fast:
                    # drain PSUM immediately (frees the acc slot in ~0.7us);
                    # stats/scale run from SBUF afterwards
                    nc.vector.tensor_copy(raw[:, ih * 512:(ih + 1) * 512], pp[:])
            u = sm.tile([128, 1], f32, tag="u", bufs=4)
            if fast:
                # sum-of-squares via ACT Square+accumulator: ACT is idle
                # before the first exp, and this shortens the serial DVE
                # chain on the startup critical path
                SQF = mybir.ActivationFunctionType.Square
                junk2 = sm.tile([128, 512], f16, tag="junk2", bufs=2)
                us = sm.tile([128, 2], f32, tag="us", bufs=2)
                for ih in range(2):
                    nc.scalar.activation(out=junk2[:], in_=pps[ih][:], func=SQF,
                                         accum_out=us[:, ih:ih + 1])
                nc.vector.tensor_scalar(u[:], us[:, 0:1], us[:, 1:2], None,
                                        alu.add)
            else:
                st2 = sm.tile([128, 2, 6], f32, tag="st2", bufs=4)
                mv = sm.tile([128, 2], f32, tag="mv", bufs=4)
                for sub in range(2):
                    nc.vector.bn_stats(out=st2[:, sub, :],
                                       in_=raw[:, sub * 512:(sub + 1) * 512])
                nc.vector.bn_aggr(out=mv[:], in_=st2[:])
                nc.vector.scalar_tensor_tensor(u[:], mv[:, 0:1], mv[:, 0:1],
                                               mv[:, 1:2], alu.mult, alu.add)
            sc = sm.tile([128, 1], f32, tag="sc", bufs=4)
            _quake_rsqrt(nc, sm, u[:], sc[:],
                         1.0 if fast else 1.0 / float(np.sqrt(N)))
            if fast:
                # drain on ACT (idle before the first exp): Copy with AP scale
                CPY = mybir.ActivationFunctionType.Copy
                for ih in range(2):
                    nc.scalar.activation(out=dst[:, mc, ih * 512:(ih + 1) * 512],
                                         in_=pps[ih][:], func=CPY,
                                         scale=sc[:, 0:1])
            else:
                nc.gpsimd.tensor_scalar(dst[:, mc, :], raw[:], sc[:, 0:1], None, alu.mult)

        def proj_v(nb, jcs):
            for jc in jcs:
                vp = ps.tile([128, 512], f32, tag="acc", bufs=2)
                for kc in range(2):
                    nc.tensor.matmul(
                        vp[:, 0:HID],
                        yts[nb][:, kc, jc * 128:(jc + 1) * 128],
                        wv_sb[:, kc, :],
                        start=(kc == 0), stop=(kc == 1))
                vt = big.tile([128, 4, 65], f16, tag="vt", bufs=16)
                nc.vector.tensor_copy(vt[:, :, 0:64],
                                      vp[:, 0:HID].rearrange("p (h d) -> p h d", h=4))
                nc.gpsimd.memset(vt[:, :, 64:65], 1.0)
                vts[nb].append(vt)

        def stexp(nb, h, jc):
            """S_T + exp for one (head, jc) tile; returns the et tile."""
            hp, hr = h // 2, 64 * (h % 2)
            qn, kn = qns[nb], kns[nb]
            st = ps.tile([128, N], f32, tag="st", bufs=2)
            for ih in range(2):
                nc.tensor.matmul(
                    st[:, ih * 512:(ih + 1) * 512],
                    kn[hr:hr + 64, hp, jc * 128:(jc + 1) * 128],
                    qn[hr:hr + 64, hp, ih * 512:(ih + 1) * 512],
                    start=True, stop=True)
            et = big.tile([128, N], f16, tag="et", bufs=26)
            if jc in POLY_JCS:
                # et = (SQ*(10*sim + PH))^2 + PK  (deg-2 exp fit)
                t16 = big.tile([128, N], f16, tag="t16", bufs=2)
                u16 = big.tile([128, N], f16, tag="u16", bufs=2)
                nc.vector.tensor_scalar(t16[:], st[:], 10.0 * SQ, SQ * PH,
                                        alu.mult, alu.add)
                nc.gpsimd.tensor_tensor(u16[:], t16[:], t16[:], alu.mult)
                nc.gpsimd.tensor_scalar(et[:], u16[:], PK, None, alu.add)
            else:
                nc.scalar.activation(out=et[:], in_=st[:], func=EXP, scale=10.0)
            return et

        def pv_chunk(nb, h, ets, ota, otb, ics):
            # PV: ic-outer so each PSUM accumulation group (over jc) completes
            # before the next one starts — start=True clears the whole bank's
            # has_written bits, so interleaved groups in a bank lose their
            # first contribution.  Col 64 of each 65-block (from vt's ones
            # column) accumulates the softmax denominator.
            for ic in ics:
                ot = ota if ic < 4 else otb
                for jc in range(8):
                    nc.tensor.matmul(
                        ot[:, ic % 4, :],
                        ets[jc][:, ic * 128:(ic + 1) * 128],
                        vts[nb][jc][:, h, :],
                        start=(jc == 0), stop=(jc == 7))

        def norm_head(nb, h, ota, otb, act=False):
            r8 = sm.tile([128, 8], f32, tag="r8", bufs=4)
            nc.vector.reciprocal(r8[:, 0:4], ota[:, :, 64])
            nc.vector.reciprocal(r8[:, 4:8], otb[:, :, 64])
            oraw = big.tile([128, 8, 64], f16, tag="oraw", bufs=2)
            if act:
                nc.scalar.copy(oraw[:, 0:4, :], ota[:, :, 0:64])
                nc.scalar.copy(oraw[:, 4:8, :], otb[:, :, 0:64])
            else:
                nc.vector.tensor_copy(oraw[:, 0:4, :], ota[:, :, 0:64])
                nc.vector.tensor_copy(oraw[:, 4:8, :], otb[:, :, 0:64])
            on = onorms[nb]
            for ic in range(8):
                nc.gpsimd.tensor_scalar(on[:, ic, h * 64:(h + 1) * 64],
                                        oraw[:, ic, :], r8[:, ic:ic + 1],
                                        None, alu.mult)

        def trans_o(nb, kcs=(0, 1), act=False):
            on, otr = onorms[nb], otrs[nb]
            for kc in kcs:
                for ih in range(2):
                    tp = ps.tile([128, 512], f32, tag="acc", bufs=2)
                    for qt in range(4):
                        ic = ih * 4 + qt
                        nc.tensor.matmul(
                            tp[:, qt * 128:(qt + 1) * 128],
                            on[:, ic, kc * 128:(kc + 1) * 128],
                            id_sb[:],
                            start=True, stop=True)
                    if act:
                        nc.scalar.copy(otr[:, kc, ih * 512:(ih + 1) * 512], tp[:])
                    else:
                        nc.vector.tensor_copy(otr[:, kc, ih * 512:(ih + 1) * 512], tp[:])

        def zproj(nb, act=False):
            IDENT = mybir.ActivationFunctionType.Identity
            otr = otrs[nb]
            for mc in range(2):
                zs = big.tile([128, N], f32, tag="zs", bufs=2)
                for ih in range(2):
                    zp = ps.tile([128, 512], f32, tag="acc", bufs=2)
                    for kc in range(2):
                        nc.tensor.matmul(
                            zp[:],
                            wo_sb[:, kc, mc * 128:(mc + 1) * 128],
                            otr[:, kc, ih * 512:(ih + 1) * 512],
                            start=(kc == 0), stop=(kc == 1))
                    if act:
                        nc.scalar.activation(out=zs[:, ih * 512:(ih + 1) * 512],
                                             in_=zp[:], func=IDENT,
                                             bias=b_sb[:, mc, 0:1], scale=1.0)
                    else:
                        nc.vector.tensor_scalar(zs[:, ih * 512:(ih + 1) * 512], zp[:],
                                                b_sb[:, mc, 0:1], None, alu.add)
                    nc.sync.dma_start(
                        out=out[nb, mc * 128:(mc + 1) * 128, ih * 512:(ih + 1) * 512],
                        in_=zs[:, ih * 512:(ih + 1) * 512])

        # ---- software-pipelined emission: (st+exp) of head slot s runs
        # while PV of slot s-1 accumulates; projections/transposes/zproj of
        # the other batch ride between slots. ----
        HEAD_SEQ = [(0, 0), (0, 1), (0, 2), (0, 3), (1, 0), (1, 1), (1, 2), (1, 3)]
        # extra work interleaved after each slot's st/exp stream (one
        # projection per slot so PSUM-drain latency never queues up):
        PRE = {
            0: lambda: proj_one(0, 1, wk_sb, yts[0], kns[0]),
            1: lambda: proj_one(0, 1, wq_sb, xts[0], qns[0]),
            2: lambda: proj_one(1, 0, wk_sb, yts[1], kns[1]),
            3: lambda: (proj_v(1, range(0, 4)),
                        proj_one(1, 0, wq_sb, xts[1], qns[1])),
            4: lambda: (proj_v(1, range(4, 8)),
                        proj_one(1, 1, wk_sb, yts[1], kns[1])),
            5: lambda: (proj_one(1, 1, wq_sb, xts[1], qns[1]),
                        trans_o(0)),
            6: lambda: zproj(0),
            7: lambda: trans_o(1, kcs=(0,)),
        }
        proj_one(0, 0, wk_sb, yts[0], kns[0], fast=True)
        proj_one(0, 0, wq_sb, xts[0], qns[0], fast=True)
        prev = None  # (nb, h, ets, ot, dn)
        for s, (nb, h) in enumerate(HEAD_SEQ):
            ets = []
            for jc in range(8):
                ets.append(stexp(nb, h, jc))
                if s == 0 and jc == 1:
                    proj_v(0, range(8))
                # weave the previous head's PV between st pairs (coarse
                # chunks so a ring-stalled st never blocks ready PV work
                # for long, but ACT stays fed across the slot boundary)
                if prev is not None and jc == 1:
                    pv_chunk(*prev, ics=range(0, 4))
                if prev is not None and jc == 3:
                    pv_chunk(*prev, ics=range(4, 8))
                if prev is not None and jc == 4:
                    norm_head(prev[0], prev[1], prev[3], prev[4])
            ota = ps.tile([128, 4, 65], f32, tag="ota", bufs=1, name=f"ota{s}")
            otb = ps.tile([128, 4, 65], f32, tag="otb", bufs=1, name=f"otb{s}")
            prev = (nb, h, ets, ota, otb)
            if s in PRE:
                PRE[s]()
        pv_chunk(*prev, ics=range(8))
        norm_head(prev[0], prev[1], prev[3], prev[4], act=True)
        trans_o(1, kcs=(1,), act=True)
        zproj(1)
        del onec

    nc.finalize()
    return nc


def _get_nc():
    if "nc" not in _CACHE:
        _CACHE["nc"] = _build_nc()
    return _CACHE["nc"]


def kernel(x, y, w_qkv, w_out, b_out):
    from concourse.bass_utils import run_bass_kernel_spmd

    nc = _get_nc()

    x = np.asarray(x, dtype=np.float32).reshape(16, C, N).astype(np.float16)
    y = np.asarray(y, dtype=np.float32).reshape(16, C, N).astype(np.float16)
    w_qkv = np.asarray(w_qkv, dtype=np.float32)
    wq_t = np.ascontiguousarray(w_qkv[0:HID].T).astype(np.float16)
    wk_t = np.ascontiguousarray(w_qkv[HID:2 * HID].T).astype(np.float16)
    wv_t = np.ascontiguousarray(w_qkv[2 * HID:3 * HID].T).astype(np.float16)
    wo_t = np.ascontiguousarray(np.asarray(w_out, dtype=np.float32).T).astype(np.float16)
    bo = np.ascontiguousarray(
        np.asarray(b_out, dtype=np.float32).reshape(2, 128, 1))
    ident = np.eye(128, dtype=np.float16)

    in_maps = []
    for c in range(NCORES):
        in_maps.append({
            "x": np.ascontiguousarray(x[c * NB:(c + 1) * NB]),
            "y": np.ascontiguousarray(y[c * NB:(c + 1) * NB]),
            "wq_t": wq_t, "wk_t": wk_t, "wv_t": wv_t, "wo_t": wo_t,
            "b_out": bo, "ident": ident,
        })

    res = run_bass_kernel_spmd(nc, in_maps, list(range(NCORES)))
    full = np.concatenate([res.results[i]["out"] for i in range(NCORES)], axis=0)
    return full.reshape(16, C, 32, 32)
